# revision 9
# baseline (speedup 1.0000x reference)
"""Trainium2 Bass kernel for L0-regularized linear forward (hard-concrete gate).

Computes out[b,o] = sum_i x[b,i] * W[o,i] * z[b,o,i] + bias[o]
  where s = sigmoid((log(u) - log1p(-u) + log_alpha) / (2/3))
        z = clip(s * 1.2 - 0.1, 0, 1)

Shapes: x[32,2048] u[32,2048,2048] W[2048,2048] la[2048,2048] bias[2048]
Sharding: output-dim sharded, 2048/8 = 256 rows per core (each core reads
its slice of u/W/la/bias + full x; no collectives; concat outputs on host).

Per-core pipeline (o-tile layout [128 part, 2 halves, 2048 free]):
  ACT:  L1 = ln(u); L1 -= ln(1-u) via DVE; t = L1 + la (POOL); s = sigmoid(1.5 t)
  DVE:  z = clamp(1.2 s - 0.1, 0, 1); p = z * x_bcast;
        tensor_tensor_reduce: acc[o] = bias[o] + sum_i p * W   -> DMA to HBM
ACT table sets (ln vs sigmoid) are batched in groups of B_GROUP batches to
amortize the ~2.7us table switch.
"""

import sys
from contextlib import ExitStack

import numpy as np

if "/opt/trn_rl_repo" not in sys.path:
    sys.path.insert(0, "/opt/trn_rl_repo")

import concourse.bass as bass
import concourse.tile as tile
from concourse import bacc, mybir
from concourse.bass_utils import run_bass_kernel_spmd

F32 = mybir.dt.float32
F16 = mybir.dt.float16

B, OUT, IN = 32, 2048, 2048
N_CORES = 8
O_SH = OUT // N_CORES          # 256 output rows per core
H = O_SH // 128                # 2 partition-halves per core
B_GROUP = 8                    # batches per ACT-table-set phase

_CACHE = {}


def _build_nc(trace=False):
    key = ("nc", trace)
    if key in _CACHE:
        return _CACHE[key]

    nc = bacc.Bacc(
        "TRN2",
        target_bir_lowering=False,
        debug=False,
        num_devices=N_CORES,
    )
    x_d = nc.dram_tensor("x", [B, IN], F32, kind="ExternalInput").ap()
    u_d = nc.dram_tensor("u", [B, O_SH, IN], F32, kind="ExternalInput").ap()
    w_d = nc.dram_tensor("w", [O_SH, IN], F32, kind="ExternalInput").ap()
    la_d = nc.dram_tensor("la", [O_SH, IN], F32, kind="ExternalInput").ap()
    bias_d = nc.dram_tensor("bias", [O_SH], F32, kind="ExternalInput").ap()
    out_d = nc.dram_tensor("out", [B, O_SH], F32, kind="ExternalOutput").ap()

    with TileCtx(nc) as tc, ExitStack() as ctx:
        _kernel_body(ctx, tc, x_d, u_d, w_d, la_d, bias_d, out_d)

    nc.compile()
    _CACHE[key] = nc
    return nc


def TileCtx(nc):
    return tile.TileContext(nc)


def _bcast_row(ap_row):
    """[1, n] AP -> [128, n] AP with 0 partition stride."""
    return bass.AP(
        tensor=ap_row.tensor,
        offset=ap_row.offset,
        ap=[[0, 128], list(ap_row.ap[-1])],
    )


def _kernel_body(ctx, tc, x_d, u_d, w_d, la_d, bias_d, out_d):
    nc = tc.nc
    Ln = mybir.ActivationFunctionType.Ln
    Sig = mybir.ActivationFunctionType.Sigmoid
    op = mybir.AluOpType

    singles = ctx.enter_context(tc.tile_pool(name="singles", bufs=1))

    # --- constants: W, la as f16 [128, H, IN]; bias cols; x16 rows ---
    w16 = singles.tile([128, H, IN], F16)
    la16 = singles.tile([128, H, IN], F16)
    with tc.tile_pool(name="setup", bufs=1) as setup:
        w32 = setup.tile([128, H, IN], F32)
        nc.sync.dma_start(out=w32, in_=w_d.rearrange("(h p) i -> p h i", p=128))
        nc.vector.tensor_copy(w16, w32)
        la32 = setup.tile([128, H, IN], F32)
        nc.sync.dma_start(out=la32, in_=la_d.rearrange("(h p) i -> p h i", p=128))
        nc.vector.tensor_copy(la16, la32)

    x16_hbm = nc.dram_tensor("x16tmp", [B, IN], F16, kind="Internal").ap()
    with tc.tile_pool(name="setup2", bufs=1) as setup:
        x32 = setup.tile([B, IN], F32)
        nc.sync.dma_start(out=x32, in_=x_d)
        x16 = setup.tile([B, IN], F16)
        nc.vector.tensor_copy(x16, x32)
        nc.sync.dma_start(out=x16_hbm, in_=x16)

    bias_col = singles.tile([128, H], F32)
    nc.sync.dma_start(out=bias_col, in_=bias_d.rearrange("(h p) -> p h", p=128))

    # --- pools for the main loop ---
    upool = ctx.enter_context(tc.tile_pool(name="u", bufs=2))
    l1pool = ctx.enter_context(tc.tile_pool(name="l1", bufs=2))
    l2pool = ctx.enter_context(tc.tile_pool(name="l2", bufs=2))
    tpool = ctx.enter_context(tc.tile_pool(name="t", bufs=B_GROUP + 1))
    zpool = ctx.enter_context(tc.tile_pool(name="z", bufs=3))
    xbpool = ctx.enter_context(tc.tile_pool(name="xb", bufs=3))
    ppool = ctx.enter_context(tc.tile_pool(name="p", bufs=4))
    apool = ctx.enter_context(tc.tile_pool(name="acc", bufs=8))

    out_v = out_d.rearrange("b (h p) -> b p h", p=128)

    for g0 in range(0, B, B_GROUP):
        grp = range(g0, min(g0 + B_GROUP, B))
        t_tiles = {}
        # ---- phase 1: natural_log table set ----
        for b in grp:
            ut = upool.tile([128, H, IN], F32)
            nc.sync.dma_start(
                out=ut, in_=u_d[b].rearrange("(h p) i -> p h i", p=128)
            )
            l1 = l1pool.tile([128, H, IN], F16)
            nc.scalar.activation(l1, ut, Ln)                      # ln(u)
            l2 = l2pool.tile([128, H, IN], F16)
            nc.scalar.activation(l2, ut, Ln, bias=1.0, scale=-1.0)  # ln(1-u)
            nc.vector.tensor_sub(l1, l1, l2)                      # logit(u), in place
            t16 = tpool.tile([128, H, IN], F16)
            nc.gpsimd.tensor_add(t16, l1, la16)                   # + log_alpha
            t_tiles[b] = t16
        # ---- phase 2: sigmoid table set ----
        for b in grp:
            t16 = t_tiles[b]
            nc.scalar.activation(t16, t16, Sig, scale=1.5)        # s, in place
            z16 = zpool.tile([128, H, IN], F16)
            nc.vector.tensor_scalar(z16, t16, 1.2, -0.1, op.mult, op.add)
            nc.vector.tensor_scalar(z16, z16, 0.0, 1.0, op.max, op.min)
            xb = xbpool.tile([128, IN], F16)
            nc.sync.dma_start(out=xb, in_=_bcast_row(x16_hbm[b : b + 1, :]))
            for h in range(H):
                p16 = ppool.tile([128, IN], F16)
                nc.vector.tensor_mul(p16, z16[:, h, :], xb)
                acc = apool.tile([128, 1], F32)
                nc.vector.scalar_tensor_tensor(
                    out=p16,
                    in0=p16,
                    scalar=1.0,
                    in1=w16[:, h, :],
                    op0=op.bypass,
                    op1=op.mult,
                    accum_out=acc,
                )
                nc.vector.tensor_add(acc, acc, bias_col[:, h : h + 1])
                nc.sync.dma_start(out=out_v[b, :, h : h + 1], in_=acc)


def kernel(x, u, weight, log_alpha, bias):
    x = np.ascontiguousarray(x, dtype=np.float32)
    u = np.ascontiguousarray(u, dtype=np.float32)
    weight = np.ascontiguousarray(weight, dtype=np.float32)
    log_alpha = np.ascontiguousarray(log_alpha, dtype=np.float32)
    bias = np.ascontiguousarray(bias, dtype=np.float32)

    nc = _build_nc()

    in_maps = []
    for c in range(N_CORES):
        sl = slice(c * O_SH, (c + 1) * O_SH)
        in_maps.append(
            {
                "x": x,
                "u": np.ascontiguousarray(u[:, sl, :]),
                "w": np.ascontiguousarray(weight[sl]),
                "la": np.ascontiguousarray(log_alpha[sl]),
                "bias": np.ascontiguousarray(bias[sl]),
            }
        )

    import os

    trace = bool(int(os.environ.get("KERNEL_TRACE", "0")))
    res = run_bass_kernel_spmd(
        nc, in_maps, core_ids=list(range(N_CORES)), trace=trace
    )
    kernel._last = res

    out = np.empty((B, OUT), dtype=np.float32)
    for c in range(N_CORES):
        out[:, c * O_SH : (c + 1) * O_SH] = res.results[c]["out"]
    return out
